# revision 2
# baseline (speedup 1.0000x reference)
"""Trainium2 Bass kernel for the reaction-wheel encoder elementwise problem (v3).

Reference semantics (per element, f32 unless noted):
    temp   = wheel_speeds * K + remaining_clicks        (K = DT * CPR, f32)
    clicks = trunc(temp)
    nominal_out = clicks * (1/K)
    nominal_rem = temp - clicks
    state == 0 (nominal): out = nominal_out, rem = nominal_rem
    state == 1 (off):     out = 0,           rem = 0
    state == 2 (stuck):   out = converted,   rem = remaining_clicks

v3 design (vs the 141.6us baseline):
  * HBM traffic 21 -> 15 B/elem: `converted` sent as bf16, outputs
    written as bf16 (pure relative rounding of final values; tolerance
    2e-2 >> bf16's 2^-9).
  * Measured HW op costs per [128,2048] tile: custom DVE op 2132ns,
    tensor_tensor 2054, copy_predicated(2fd) 4108, ACT activation 1812,
    Pool tensor_tensor 4195 (0.42 eff).  scalar_tensor_tensor measured
    2623ns (cost model's "2x_2p" f32 mode is NOT real) - avoided.
  * Concurrent GpSimd/Pool activity was measured to slow concurrent DVE
    custom ops 3-4x (SBUF contention from the Q7 software engine), so
    Pool is left idle.
  * The two outputs are stored as INTERLEAVED bf16 pairs [rem_i|out_i]
    so the stuck override is ONE fd-wide copy_predicated on the i32
    pair view (copy_predicated has no 16-bit fast mode; element count
    is what costs).  The override data [rc_bf16|cv_bf16] arrives from
    the host as one interleaved i32 plane, and the i32 mask is a single
    ACT op.  Work split:
      DVE : A temp=affine(ws,rc,K); B x=temp*m0 (tensor_tensor);
            C rem=REM_TRUNC(x,sgn) f32; D out=CLICKS_SCALE(x,rem)->ob
            bf16 (strided); PRED copy_predicated(ob_i32, m2_i32, pd_i32)
      ACT : m0=Relu(1-st) i8; m2=Relu(st-1) i32; sgn=Sign(temp) i8
            (masked lanes have d=0 so the unmasked sign is safe);
            CONV rem->ob strided bf16
  * Masked lanes: x=temp*m0 collapses to +-0 through the trunc chain so
    off lanes yield +-0 in both outputs; PRED fixes stuck lanes.

Layout per core: packed_main [nt,P,9fd] u8 rows = ws f32 | rc f32 |
st i8; packed_pd [nt,P,fd] i32 = (cv_bf16<<16)|rc_bf16; packed_out
[nt,P,fd] i32 = interleaved (rem_bf16, out_bf16) pairs.

trunc(x): rn=(x+1.5*2^23)-1.5*2^23 (RNE); d=x-rn; rem=d+(d*sgn<0)*sgn.
"""

import os
import sys

import numpy as np
import ml_dtypes

for _p in ("/opt/trn_rl_repo", os.path.expanduser("~/.axon_site/_ro/trn_rl_repo")):
    if os.path.isdir(_p) and _p not in sys.path:
        sys.path.insert(0, _p)

import concourse.bass as bass
import concourse.mybir as mybir
import concourse.dve_ops as dve_ops
from concourse.dve_spec import C0 as _C0
from concourse.dve_spec import Spec, Src0, Src1, Zero, lower, _has_src1
from concourse.dve_uop import DveOpSpec
from concourse.bass_utils import run_bass_kernel_spmd

N_TOTAL = 16_777_216
N_CORES = 8
PER_CORE = N_TOTAL // N_CORES  # 2,097,152
P = 128
FD = 2048
NT = PER_CORE // (P * FD)  # 8 tiles/core
BUFS_IN = 4

F32 = mybir.dt.float32
BF16 = mybir.dt.bfloat16
I8 = mybir.dt.int8
I16 = mybir.dt.int16
I32 = mybir.dt.int32
U8 = mybir.dt.uint8
ALU = mybir.AluOpType
ACT = mybir.ActivationFunctionType

K32 = np.float32(0.1 * (2048.0 / (2.0 * np.pi)))
INVK32 = np.float32(1.0) / K32
MAGIC = float(np.float32(1.5 * 2.0**23))


def _register_custom_op(name, spec):
    for op in dve_ops.OPS:
        if op.name == name:
            return op
    row = dve_ops._CUSTOM_DVE_ROW_BASE + len(dve_ops.OPS)
    assert row < 0x20
    dve_ops._SUB_OPCODE_FOR_NAME[name] = row
    shas = {}
    for ver in ("v3", "v4"):
        try:
            tmp = DveOpSpec(
                name=name, opcode=row, uops=lower(spec, ver=ver),
                rd1_en=_has_src1(spec),
            )
            shas[ver] = tmp.sha(ver)
        except Exception:
            pass
    op = dve_ops.DveOp(name, spec, subdim=False, uops_sha=shas)
    dve_ops.OPS.append(op)
    dve_ops.CUSTOM_DVE_SPECS[name] = spec
    return op


def _rem_trunc_ref(in0, in1, s0, s1, imm2):
    x = in0.astype(np.float32)
    sgn = in1.astype(np.float32)
    rn = ((x + np.float32(s0)) - np.float32(s0)).astype(np.float32)
    d = (x - rn).astype(np.float32)
    away = ((d * sgn).astype(np.float32) < 0).astype(np.float32)
    return (d + away * sgn).astype(np.float32)


_d = (Src0 - ((Src0 + _C0) - _C0))
REM_TRUNC = _register_custom_op(
    "REM_TRUNC_ANT",
    Spec(body=_d + ((_d * Src1) < Zero) * Src1, reference=_rem_trunc_ref),
)

CLICKS_SCALE = _register_custom_op(
    "CLICKS_SCALE_ANT",
    Spec(
        body=(Src0 - Src1) * _C0,
        reference=lambda in0, in1, s0, s1, imm2: (
            (in0.astype(np.float32) - in1.astype(np.float32)) * np.float32(s0)
        ).astype(np.float32),
    ),
)


def build_nc(nt: int = NT, fd: int = FD, splits=None) -> bass.Bass:
    if splits is None:
        splits = [2, 2, 1, 1, 1, 1, 2, 2]  # halves at both ends x2
    nc = bass.Bass()
    in_d = nc.dram_tensor("packed_main", [nt, P, 9 * fd], U8, kind="ExternalInput")
    pd_d = nc.dram_tensor("packed_pd", [nt, P, fd], I32, kind="ExternalInput")
    out_d = nc.dram_tensor("packed_out", [nt, P, fd], I32, kind="ExternalOutput")
    in_v, pd_v, out_v = in_d[:], pd_d[:], out_d[:]

    with nc.sbuf_tensor("t_in", [P, BUFS_IN, 9 * fd], U8) as t_in, \
         nc.sbuf_tensor("t_pd", [P, 4, fd], I32) as t_pd, \
         nc.sbuf_tensor("t_temp", [P, 2, fd], F32) as t_temp, \
         nc.sbuf_tensor("t_x", [P, 2, fd], F32) as t_x, \
         nc.sbuf_tensor("t_sgn", [P, 2, fd], I8) as t_sgn, \
         nc.sbuf_tensor("t_rem", [P, 2, fd], F32) as t_rem, \
         nc.sbuf_tensor("t_m0", [P, 2, fd], I8) as t_m0, \
         nc.sbuf_tensor("t_m2", [P, 3, fd], I8) as t_m2, \
         nc.sbuf_tensor("t_ob", [P, 3, 2 * fd], BF16) as t_ob, \
         nc.sbuf_tensor("t_neg1", [P, 1], F32) as t_neg1:
        s_in1 = [nc.semaphore(name=f"s_in1_{b}").__enter__() for b in range(BUFS_IN)]
        s_in2 = [nc.semaphore(name=f"s_in2_{b}").__enter__() for b in range(4)]
        s_out = [nc.semaphore(name=f"s_out{b}").__enter__() for b in range(3)]
        s_dve = nc.semaphore(name="s_dve").__enter__()
        s_act = nc.semaphore(name="s_act").__enter__()
        s_ini = nc.semaphore(name="s_ini").__enter__()

        sched = []
        for t in range(nt):
            k = splits[t]
            w = fd // k
            for j in range(k):
                sched.append((t, j * w, w))
        nv = len(sched)
        # Ticks per virtual iteration (DVE order A, C, D, B, PRED —
        # no drain/tick after A and D; B's tick covers A's completion):
        #   DVE (iters 0..nv+1, 3/iter): C(i-1)=3i+1, B(i)=3i+2,
        #       PRED(i-2)=3i+3
        #   ACT (iters 0..nv, 4/iter): m0(v)=4v+1, m2(v)=4v+2,
        #       sgn(v)=4v+3, conv(v-1)=4v+4
        ka = [0] * nv
        kb = [0] * nv
        cnt1 = [0] * BUFS_IN
        cnt2 = [0] * 4

        def dma_in1(v):
            t, c, w = sched[v]
            b = v % BUFS_IN
            if w == fd:
                nc.sync.dma_start(
                    t_in.ap()[:, b, :], in_v[t]
                ).then_inc(s_in1[b], 16)
                cnt1[b] += 1
            else:
                # ws+rc column chunk: two 4w-byte ranges at stride 4*fd
                src = in_v[t, :, 0 : 8 * fd].rearrange(
                    "p (a z) -> p a z", a=2
                )[:, :, 4 * c : 4 * c + 4 * w]
                dst = t_in.ap()[:, b, 0 : 8 * fd].rearrange(
                    "p (a z) -> p a z", a=2
                )[:, :, 4 * c : 4 * c + 4 * w]
                nc.sync.dma_start(dst, src).then_inc(s_in1[b], 16)
                nc.sync.dma_start(
                    t_in.ap()[:, b, 8 * fd + c : 8 * fd + c + w],
                    in_v[t, :, 8 * fd + c : 8 * fd + c + w],
                ).then_inc(s_in1[b], 16)
                cnt1[b] += 2
            ka[v] = 16 * cnt1[b]

        def dma_in2(v):
            t, c, w = sched[v]
            b3 = v % 4
            nc.sync.dma_start(
                t_pd.ap()[:, b3, c : c + w],
                pd_v[t, :, c : c + w],
            ).then_inc(s_in2[b3], 16)
            cnt2[b3] += 1
            kb[v] = 16 * cnt2[b3]

        def dma_in(v):
            dma_in1(v)
            dma_in2(v)

        def in_f32(b, byte_off, w4):
            return t_in.ap()[:, b, byte_off : byte_off + 4 * w4].bitcast(F32)

        # ---- SP queue -----------------------------------------------------
        for v in range(min(BUFS_IN, nv)):
            dma_in(v)
        for v in range(nv):
            t, c, w = sched[v]
            s = v % 3
            if v + BUFS_IN < nv:
                # t_in slot (v+BUFS_IN)%4 = v%4: readers A(v) on DVE and
                # the st read on ACT (m2(v)=4v+2)
                nc.sync.wait_ge(s_dve, 3 * v + 2)   # B(v), covers A(v)
                nc.sync.wait_ge(s_act, 4 * v + 2)   # m2(v)
                dma_in1(v + BUFS_IN)
            nc.sync.wait_ge(s_dve, 3 * (v + 2) + 3)  # PRED(v) done
            if w == fd:
                dst = out_v[t]
                src = t_ob.ap()[:, s].bitcast(I32)
            else:
                dst = out_v[t][:, c : c + w]
                src = t_ob.ap()[:, s].bitcast(I32)[:, c : c + w]
            nc.sync.dma_start(dst, src).then_inc(s_out[s], 16)
            if v + BUFS_IN < nv:
                dma_in2(v + BUFS_IN)

        # ---- DVE queue ----------------------------------------------------
        nc.vector.memset(t_neg1.ap(), -1.0)
        nc.vector.drain()
        nc.vector.nop().then_inc(s_ini, 1)
        for i in range(nv + 2):
            # A(i): temp = (ws*K) + rc   (no drain: C/D don't read temp,
            # and B(i) later in this iteration carries the tick)
            if i < nv:
                t, c, w = sched[i]
                si = i % BUFS_IN
                nc.vector.wait_ge(s_in1[si], ka[i])
                if i >= 2:
                    nc.vector.wait_ge(s_act, 4 * (i - 2) + 3)   # sgn(i-2)
                nc.vector.affine_then_add(
                    out=t_temp.ap()[:, i % 2, 0:w],
                    in0=in_f32(si, 4 * c, w),
                    in1=in_f32(si, 4 * fd + 4 * c, w),
                    scale=float(K32), bias=0.0,
                )
            j = i - 1
            if 0 <= j < nv:
                t, c, w = sched[j]
                # C(j): rem = REM_TRUNC(x, sgn)
                nc.vector.wait_ge(s_act, 4 * j + 3)   # sgn(j)
                nc.vector._custom_dve(
                    REM_TRUNC, out=t_rem.ap()[:, j % 2, 0:w],
                    in0=t_x.ap()[:, j % 2, 0:w],
                    in1=t_sgn.ap()[:, j % 2, 0:w], s0=MAGIC,
                )
                nc.vector.drain()
                nc.vector.nop().then_inc(s_dve, 1)  # 3i+1
                # D(j): out = CLICKS_SCALE(x, rem) -> ob bf16 pairs (no drain)
                if j >= 3:
                    nc.vector.wait_ge(s_out[j % 3], 16 * (j // 3))
                nc.vector._custom_dve(
                    CLICKS_SCALE,
                    out=t_ob.ap()[:, j % 3].rearrange(
                        "p (z a) -> p z a", a=2
                    )[:, c : c + w, 1],
                    in0=t_x.ap()[:, j % 2, 0:w],
                    in1=t_rem.ap()[:, j % 2, 0:w], s0=float(INVK32),
                )
            else:
                nc.vector.nop().then_inc(s_dve, 1)  # 3i+1
            # B(i): x = temp * m0
            if i < nv:
                t, c, w = sched[i]
                nc.vector.wait_ge(s_act, 4 * i + 1)  # m0(i)
                nc.vector.tensor_tensor(
                    out=t_x.ap()[:, i % 2, 0:w],
                    in0=t_temp.ap()[:, i % 2, 0:w],
                    in1=t_m0.ap()[:, i % 2, 0:w],
                    op=ALU.mult,
                )
                nc.vector.drain()
            nc.vector.nop().then_inc(s_dve, 1)  # 3i+2
            k = i - 2
            if 0 <= k < nv:
                t, c, w = sched[k]
                # PRED(k): stuck override on the i32 pair view
                nc.vector.wait_ge(s_act, 4 * (k + 1) + 4)  # conv(k)
                nc.vector.wait_ge(s_in2[k % 4], kb[k])  # pd(k)
                nc.vector.copy_predicated(
                    out=t_ob.ap()[:, k % 3].bitcast(I32)[:, c : c + w],
                    mask=t_m2.ap()[:, k % 3, c : c + w],
                    data=t_pd.ap()[:, k % 4, c : c + w],
                )
                nc.vector.drain()
            nc.vector.nop().then_inc(s_dve, 1)  # 3i+3

        # ---- ACT queue ----------------------------------------------------
        nc.scalar.wait_ge(s_ini, 1)
        for v in range(nv + 1):
            if v < nv:
                t, c, w = sched[v]
                si = v % BUFS_IN
                st = t_in.ap()[:, si, 8 * fd + c : 8 * fd + c + w].bitcast(I8)
                nc.scalar.wait_ge(s_in1[si], ka[v])
                if v >= 2:
                    nc.scalar.wait_ge(s_dve, 3 * (v - 2) + 2)  # B(v-2): m0 slot
                # m0(v) i8
                nc.scalar.activation(
                    t_m0.ap()[:, v % 2, 0:w], st, ACT.Relu, bias=1.0, scale=-1.0
                )
                nc.scalar.drain()
                nc.scalar.nop().then_inc(s_act, 1)  # 4v+1
                # m2(v) i32
                if v >= 3:
                    nc.scalar.wait_ge(s_dve, 3 * (v - 1) + 3)  # PRED(v-3): m2 slot
                nc.scalar.activation(
                    t_m2.ap()[:, v % 3, c : c + w],
                    st, ACT.Relu, bias=t_neg1.ap(), scale=1.0,
                )
                nc.scalar.drain()
                nc.scalar.nop().then_inc(s_act, 1)  # 4v+2
                # sgn(v) from temp
                nc.scalar.wait_ge(s_dve, 3 * v + 2)  # B(v) (temp final after A)
                nc.scalar.activation(
                    t_sgn.ap()[:, v % 2, 0:w], t_temp.ap()[:, v % 2, 0:w],
                    ACT.Sign, bias=0.0, scale=1.0,
                )
                nc.scalar.drain()
                nc.scalar.nop().then_inc(s_act, 1)  # 4v+3
            else:
                for _ in range(3):
                    nc.scalar.nop().then_inc(s_act, 1)
            u = v - 1
            if 0 <= u < nv:
                t, c, w = sched[u]
                # conv(u): rem f32 -> ob strided bf16 (pair slot 0)
                nc.scalar.wait_ge(s_dve, 3 * v + 1)  # C(u) emitted in DVE iter v
                if u >= 3:
                    nc.scalar.wait_ge(s_out[u % 3], 16 * (u // 3))
                nc.scalar.activation(
                    t_ob.ap()[:, u % 3].rearrange(
                        "p (z a) -> p z a", a=2
                    )[:, c : c + w, 0],
                    t_rem.ap()[:, u % 2, 0:w], ACT.Copy, bias=0.0, scale=1.0,
                )
                nc.scalar.drain()
            nc.scalar.nop().then_inc(s_act, 1)  # 4v+4

    mybir.codegen_inst_isa_subclasses(nc)
    nc.finalize()
    return nc


_NC_CACHE: bass.Bass | None = None


def _get_nc() -> bass.Bass:
    global _NC_CACHE
    if _NC_CACHE is None:
        _NC_CACHE = build_nc()
    return _NC_CACHE


def make_in_maps(wheel_speeds, remaining_clicks, converted, rw_signal_state):
    u8 = np.uint8
    ws = np.asarray(wheel_speeds, dtype=np.float32).reshape(N_CORES, NT, P, FD)
    rc = np.asarray(remaining_clicks, dtype=np.float32).reshape(N_CORES, NT, P, FD)
    rc_bf = np.asarray(remaining_clicks, dtype=np.float32).astype(
        ml_dtypes.bfloat16).view('<u2').astype('<u4')
    cv_bf = np.asarray(converted, dtype=np.float32).astype(
        ml_dtypes.bfloat16).view('<u2').astype('<u4')
    pd32 = (rc_bf | (cv_bf << 16)).view('<i4').reshape(N_CORES, NT, P, FD)
    st8 = np.asarray(rw_signal_state, dtype=np.int32).astype(np.int8)
    packed = np.concatenate(
        [
            ws.view(u8).reshape(N_CORES, NT, P, 4 * FD),
            rc.view(u8).reshape(N_CORES, NT, P, 4 * FD),
            st8.view(u8).reshape(N_CORES, NT, P, FD),
        ],
        axis=3,
    )  # [cores, nt, P, 9*FD]
    return [
        {
            "packed_main": np.ascontiguousarray(packed[c]),
            "packed_pd": np.ascontiguousarray(pd32[c]),
        }
        for c in range(N_CORES)
    ]


def unpack_results(results):
    po = np.stack(
        [np.asarray(results[c]["packed_out"]) for c in range(N_CORES)], axis=0
    )
    pairs = po.view('<u2').reshape(N_CORES, NT, P, FD, 2)
    rem = pairs[..., 0].copy().view(ml_dtypes.bfloat16).astype(np.float32)
    out = pairs[..., 1].copy().view(ml_dtypes.bfloat16).astype(np.float32)
    return out.reshape(N_TOTAL), rem.reshape(N_TOTAL)


def kernel(wheel_speeds, remaining_clicks, converted, rw_signal_state):
    nc = _get_nc()
    in_maps = make_in_maps(wheel_speeds, remaining_clicks, converted, rw_signal_state)
    res = run_bass_kernel_spmd(nc, in_maps, core_ids=list(range(N_CORES)))
    return unpack_results(res.results)


# revision 3
# speedup vs baseline: 1.0190x; 1.0190x over previous
"""Trainium2 Bass kernel for the reaction-wheel encoder elementwise problem (v3).

Reference semantics (per element, f32 unless noted):
    temp   = wheel_speeds * K + remaining_clicks        (K = DT * CPR, f32)
    clicks = trunc(temp)
    nominal_out = clicks * (1/K)
    nominal_rem = temp - clicks
    state == 0 (nominal): out = nominal_out, rem = nominal_rem
    state == 1 (off):     out = 0,           rem = 0
    state == 2 (stuck):   out = converted,   rem = remaining_clicks

v3 design (vs the 141.6us baseline):
  * HBM traffic 21 -> 15 B/elem: `converted` sent as bf16, outputs
    written as bf16 (pure relative rounding of final values; tolerance
    2e-2 >> bf16's 2^-9).
  * Measured HW op costs per [128,2048] tile: custom DVE op 2132ns,
    tensor_tensor 2054, copy_predicated(2fd) 4108, ACT activation 1812,
    Pool tensor_tensor 4195 (0.42 eff).  scalar_tensor_tensor measured
    2623ns (cost model's "2x_2p" f32 mode is NOT real) - avoided.
  * Concurrent GpSimd/Pool activity was measured to slow concurrent DVE
    custom ops 3-4x (SBUF contention from the Q7 software engine), so
    Pool is left idle.
  * The two outputs are stored as INTERLEAVED bf16 pairs [rem_i|out_i]
    so the stuck override is ONE fd-wide copy_predicated on the i32
    pair view (copy_predicated has no 16-bit fast mode; element count
    is what costs).  The override data [rc_bf16|cv_bf16] arrives from
    the host as one interleaved i32 plane, and the i32 mask is a single
    ACT op.  Work split:
      DVE : A temp=affine(ws,rc,K); B x=temp*m0 (tensor_tensor);
            C rem=REM_TRUNC(x,sgn) f32; D out=CLICKS_SCALE(x,rem)->ob
            bf16 (strided); PRED copy_predicated(ob_i32, m2_i32, pd_i32)
      ACT : m0=Relu(1-st) i8; m2=Relu(st-1) i32; sgn=Sign(temp) i8
            (masked lanes have d=0 so the unmasked sign is safe);
            CONV rem->ob strided bf16
  * Masked lanes: x=temp*m0 collapses to +-0 through the trunc chain so
    off lanes yield +-0 in both outputs; PRED fixes stuck lanes.

Layout per core: packed_main [nt,P,9fd] u8 rows = ws f32 | rc f32 |
st i8; packed_pd [nt,P,fd] i32 = (cv_bf16<<16)|rc_bf16; packed_out
[nt,P,fd] i32 = interleaved (rem_bf16, out_bf16) pairs.

trunc(x): rn=(x+1.5*2^23)-1.5*2^23 (RNE); d=x-rn; rem=d+(d*sgn<0)*sgn.
"""

import os
import sys

import numpy as np
import ml_dtypes

for _p in ("/opt/trn_rl_repo", os.path.expanduser("~/.axon_site/_ro/trn_rl_repo")):
    if os.path.isdir(_p) and _p not in sys.path:
        sys.path.insert(0, _p)

import concourse.bass as bass
import concourse.mybir as mybir
import concourse.dve_ops as dve_ops
from concourse.dve_spec import C0 as _C0
from concourse.dve_spec import Spec, Src0, Src1, Zero, lower, _has_src1
from concourse.dve_uop import DveOpSpec
from concourse.bass_utils import run_bass_kernel_spmd

N_TOTAL = 16_777_216
N_CORES = 8
PER_CORE = N_TOTAL // N_CORES  # 2,097,152
P = 128
FD = 2048
NT = PER_CORE // (P * FD)  # 8 tiles/core
BUFS_IN = 4

F32 = mybir.dt.float32
BF16 = mybir.dt.bfloat16
I8 = mybir.dt.int8
I16 = mybir.dt.int16
I32 = mybir.dt.int32
U8 = mybir.dt.uint8
ALU = mybir.AluOpType
ACT = mybir.ActivationFunctionType

K32 = np.float32(0.1 * (2048.0 / (2.0 * np.pi)))
INVK32 = np.float32(1.0) / K32
MAGIC = float(np.float32(1.5 * 2.0**23))


def _register_custom_op(name, spec):
    for op in dve_ops.OPS:
        if op.name == name:
            return op
    row = dve_ops._CUSTOM_DVE_ROW_BASE + len(dve_ops.OPS)
    assert row < 0x20
    dve_ops._SUB_OPCODE_FOR_NAME[name] = row
    shas = {}
    for ver in ("v3", "v4"):
        try:
            tmp = DveOpSpec(
                name=name, opcode=row, uops=lower(spec, ver=ver),
                rd1_en=_has_src1(spec),
            )
            shas[ver] = tmp.sha(ver)
        except Exception:
            pass
    op = dve_ops.DveOp(name, spec, subdim=False, uops_sha=shas)
    dve_ops.OPS.append(op)
    dve_ops.CUSTOM_DVE_SPECS[name] = spec
    return op


def _rem_trunc_ref(in0, in1, s0, s1, imm2):
    x = in0.astype(np.float32)
    sgn = in1.astype(np.float32)
    rn = ((x + np.float32(s0)) - np.float32(s0)).astype(np.float32)
    d = (x - rn).astype(np.float32)
    away = ((d * sgn).astype(np.float32) < 0).astype(np.float32)
    return (d + away * sgn).astype(np.float32)


_d = (Src0 - ((Src0 + _C0) - _C0))
REM_TRUNC = _register_custom_op(
    "REM_TRUNC_ANT",
    Spec(body=_d + ((_d * Src1) < Zero) * Src1, reference=_rem_trunc_ref),
)

CLICKS_SCALE = _register_custom_op(
    "CLICKS_SCALE_ANT",
    Spec(
        body=(Src0 - Src1) * _C0,
        reference=lambda in0, in1, s0, s1, imm2: (
            (in0.astype(np.float32) - in1.astype(np.float32)) * np.float32(s0)
        ).astype(np.float32),
    ),
)


def build_nc(nt: int = NT, fd: int = FD, splits=None) -> bass.Bass:
    if splits is None:
        splits = [2, 2, 1, 1, 1, 1, 1, 2]  # halves at both ends x2
    nc = bass.Bass()
    in_d = nc.dram_tensor("packed_main", [nt, P, 9 * fd], U8, kind="ExternalInput")
    pd_d = nc.dram_tensor("packed_pd", [nt, P, fd], I32, kind="ExternalInput")
    out_d = nc.dram_tensor("packed_out", [nt, P, fd], I32, kind="ExternalOutput")
    in_v, pd_v, out_v = in_d[:], pd_d[:], out_d[:]

    with nc.sbuf_tensor("t_in", [P, BUFS_IN, 9 * fd], U8) as t_in, \
         nc.sbuf_tensor("t_pd", [P, 4, fd], I32) as t_pd, \
         nc.sbuf_tensor("t_temp", [P, 2, fd], F32) as t_temp, \
         nc.sbuf_tensor("t_x", [P, 2, fd], F32) as t_x, \
         nc.sbuf_tensor("t_sgn", [P, 2, fd], I8) as t_sgn, \
         nc.sbuf_tensor("t_rem", [P, 2, fd], F32) as t_rem, \
         nc.sbuf_tensor("t_m0", [P, 2, fd], I8) as t_m0, \
         nc.sbuf_tensor("t_m2", [P, 3, fd], I8) as t_m2, \
         nc.sbuf_tensor("t_ob", [P, 3, 2 * fd], BF16) as t_ob, \
         nc.sbuf_tensor("t_neg1", [P, 1], F32) as t_neg1:
        s_in1 = [nc.semaphore(name=f"s_in1_{b}").__enter__() for b in range(BUFS_IN)]
        s_in2 = [nc.semaphore(name=f"s_in2_{b}").__enter__() for b in range(4)]
        s_out = [nc.semaphore(name=f"s_out{b}").__enter__() for b in range(3)]
        s_dve = nc.semaphore(name="s_dve").__enter__()
        s_act = nc.semaphore(name="s_act").__enter__()
        s_ini = nc.semaphore(name="s_ini").__enter__()

        sched = []
        for t in range(nt):
            k = splits[t]
            w = fd // k
            for j in range(k):
                sched.append((t, j * w, w))
        nv = len(sched)
        # Ticks per virtual iteration (DVE order A, C, D, B, PRED —
        # no drain/tick after A and D; B's tick covers A's completion):
        #   DVE (iters 0..nv+1, 3/iter): C(i-1)=3i+1, B(i)=3i+2,
        #       PRED(i-2)=3i+3
        #   ACT (iters 0..nv, 4/iter): m0(v)=4v+1, m2(v)=4v+2,
        #       sgn(v)=4v+3, conv(v-1)=4v+4
        ka = [0] * nv
        kb = [0] * nv
        cnt1 = [0] * BUFS_IN
        cnt2 = [0] * 4

        def dma_in1(v):
            t, c, w = sched[v]
            b = v % BUFS_IN
            if w == fd:
                nc.sync.dma_start(
                    t_in.ap()[:, b, :], in_v[t]
                ).then_inc(s_in1[b], 16)
                cnt1[b] += 1
            else:
                # ws+rc column chunk: two 4w-byte ranges at stride 4*fd
                src = in_v[t, :, 0 : 8 * fd].rearrange(
                    "p (a z) -> p a z", a=2
                )[:, :, 4 * c : 4 * c + 4 * w]
                dst = t_in.ap()[:, b, 0 : 8 * fd].rearrange(
                    "p (a z) -> p a z", a=2
                )[:, :, 4 * c : 4 * c + 4 * w]
                nc.sync.dma_start(dst, src).then_inc(s_in1[b], 16)
                nc.sync.dma_start(
                    t_in.ap()[:, b, 8 * fd + c : 8 * fd + c + w],
                    in_v[t, :, 8 * fd + c : 8 * fd + c + w],
                ).then_inc(s_in1[b], 16)
                cnt1[b] += 2
            ka[v] = 16 * cnt1[b]

        def dma_in2(v):
            t, c, w = sched[v]
            b3 = v % 4
            nc.sync.dma_start(
                t_pd.ap()[:, b3, c : c + w],
                pd_v[t, :, c : c + w],
            ).then_inc(s_in2[b3], 16)
            cnt2[b3] += 1
            kb[v] = 16 * cnt2[b3]

        def dma_in(v):
            dma_in1(v)
            dma_in2(v)

        def in_f32(b, byte_off, w4):
            return t_in.ap()[:, b, byte_off : byte_off + 4 * w4].bitcast(F32)

        # ---- SP queue -----------------------------------------------------
        for v in range(min(BUFS_IN, nv)):
            dma_in(v)
        for v in range(nv):
            t, c, w = sched[v]
            s = v % 3
            if v + BUFS_IN < nv:
                # t_in slot (v+BUFS_IN)%4 = v%4: readers A(v) on DVE and
                # the st read on ACT (m2(v)=4v+2)
                nc.sync.wait_ge(s_dve, 3 * v + 2)   # B(v), covers A(v)
                nc.sync.wait_ge(s_act, 4 * v + 2)   # m2(v)
                dma_in1(v + BUFS_IN)
            nc.sync.wait_ge(s_dve, 3 * (v + 2) + 3)  # PRED(v) done
            if w == fd:
                dst = out_v[t]
                src = t_ob.ap()[:, s].bitcast(I32)
            else:
                dst = out_v[t][:, c : c + w]
                src = t_ob.ap()[:, s].bitcast(I32)[:, c : c + w]
            nc.sync.dma_start(dst, src).then_inc(s_out[s], 16)
            if v + BUFS_IN < nv:
                dma_in2(v + BUFS_IN)

        # ---- DVE queue ----------------------------------------------------
        nc.vector.memset(t_neg1.ap(), -1.0)
        nc.vector.drain()
        nc.vector.nop().then_inc(s_ini, 1)
        for i in range(nv + 2):
            # A(i): temp = (ws*K) + rc   (no drain: C/D don't read temp,
            # and B(i) later in this iteration carries the tick)
            if i < nv:
                t, c, w = sched[i]
                si = i % BUFS_IN
                nc.vector.wait_ge(s_in1[si], ka[i])
                if i >= 2:
                    nc.vector.wait_ge(s_act, 4 * (i - 2) + 3)   # sgn(i-2)
                nc.vector.affine_then_add(
                    out=t_temp.ap()[:, i % 2, 0:w],
                    in0=in_f32(si, 4 * c, w),
                    in1=in_f32(si, 4 * fd + 4 * c, w),
                    scale=float(K32), bias=0.0,
                )
            j = i - 1
            if 0 <= j < nv:
                t, c, w = sched[j]
                # C(j): rem = REM_TRUNC(x, sgn)
                nc.vector.wait_ge(s_act, 4 * j + 3)   # sgn(j)
                nc.vector._custom_dve(
                    REM_TRUNC, out=t_rem.ap()[:, j % 2, 0:w],
                    in0=t_x.ap()[:, j % 2, 0:w],
                    in1=t_sgn.ap()[:, j % 2, 0:w], s0=MAGIC,
                )
                nc.vector.drain()
                nc.vector.nop().then_inc(s_dve, 1)  # 3i+1
                # D(j): out = CLICKS_SCALE(x, rem) -> ob bf16 pairs (no drain)
                if j >= 3:
                    nc.vector.wait_ge(s_out[j % 3], 16 * (j // 3))
                nc.vector._custom_dve(
                    CLICKS_SCALE,
                    out=t_ob.ap()[:, j % 3].rearrange(
                        "p (z a) -> p z a", a=2
                    )[:, c : c + w, 1],
                    in0=t_x.ap()[:, j % 2, 0:w],
                    in1=t_rem.ap()[:, j % 2, 0:w], s0=float(INVK32),
                )
            else:
                nc.vector.nop().then_inc(s_dve, 1)  # 3i+1
            # B(i): x = temp * m0
            if i < nv:
                t, c, w = sched[i]
                nc.vector.wait_ge(s_act, 4 * i + 1)  # m0(i)
                nc.vector.tensor_tensor(
                    out=t_x.ap()[:, i % 2, 0:w],
                    in0=t_temp.ap()[:, i % 2, 0:w],
                    in1=t_m0.ap()[:, i % 2, 0:w],
                    op=ALU.mult,
                )
                nc.vector.drain()
            nc.vector.nop().then_inc(s_dve, 1)  # 3i+2
            k = i - 2
            if 0 <= k < nv:
                t, c, w = sched[k]
                # PRED(k): stuck override on the i32 pair view
                nc.vector.wait_ge(s_act, 4 * (k + 1) + 4)  # conv(k)
                nc.vector.wait_ge(s_in2[k % 4], kb[k])  # pd(k)
                nc.vector.copy_predicated(
                    out=t_ob.ap()[:, k % 3].bitcast(I32)[:, c : c + w],
                    mask=t_m2.ap()[:, k % 3, c : c + w],
                    data=t_pd.ap()[:, k % 4, c : c + w],
                )
                nc.vector.drain()
            nc.vector.nop().then_inc(s_dve, 1)  # 3i+3

        # ---- ACT queue ----------------------------------------------------
        nc.scalar.wait_ge(s_ini, 1)
        for v in range(nv + 1):
            if v < nv:
                t, c, w = sched[v]
                si = v % BUFS_IN
                st = t_in.ap()[:, si, 8 * fd + c : 8 * fd + c + w].bitcast(I8)
                nc.scalar.wait_ge(s_in1[si], ka[v])
                if v >= 2:
                    nc.scalar.wait_ge(s_dve, 3 * (v - 2) + 2)  # B(v-2): m0 slot
                # m0(v) i8
                nc.scalar.activation(
                    t_m0.ap()[:, v % 2, 0:w], st, ACT.Relu, bias=1.0, scale=-1.0
                )
                nc.scalar.drain()
                nc.scalar.nop().then_inc(s_act, 1)  # 4v+1
                # m2(v) i32
                if v >= 3:
                    nc.scalar.wait_ge(s_dve, 3 * (v - 1) + 3)  # PRED(v-3): m2 slot
                nc.scalar.activation(
                    t_m2.ap()[:, v % 3, c : c + w],
                    st, ACT.Relu, bias=t_neg1.ap(), scale=1.0,
                )
                nc.scalar.drain()
                nc.scalar.nop().then_inc(s_act, 1)  # 4v+2
                # sgn(v) from temp
                nc.scalar.wait_ge(s_dve, 3 * v + 2)  # B(v) (temp final after A)
                nc.scalar.activation(
                    t_sgn.ap()[:, v % 2, 0:w], t_temp.ap()[:, v % 2, 0:w],
                    ACT.Sign, bias=0.0, scale=1.0,
                )
                nc.scalar.drain()
                nc.scalar.nop().then_inc(s_act, 1)  # 4v+3
            else:
                for _ in range(3):
                    nc.scalar.nop().then_inc(s_act, 1)
            u = v - 1
            if 0 <= u < nv:
                t, c, w = sched[u]
                # conv(u): rem f32 -> ob strided bf16 (pair slot 0)
                nc.scalar.wait_ge(s_dve, 3 * v + 1)  # C(u) emitted in DVE iter v
                if u >= 3:
                    nc.scalar.wait_ge(s_out[u % 3], 16 * (u // 3))
                nc.scalar.activation(
                    t_ob.ap()[:, u % 3].rearrange(
                        "p (z a) -> p z a", a=2
                    )[:, c : c + w, 0],
                    t_rem.ap()[:, u % 2, 0:w], ACT.Copy, bias=0.0, scale=1.0,
                )
                nc.scalar.drain()
            nc.scalar.nop().then_inc(s_act, 1)  # 4v+4

    mybir.codegen_inst_isa_subclasses(nc)
    nc.finalize()
    return nc


_NC_CACHE: bass.Bass | None = None


def _get_nc() -> bass.Bass:
    global _NC_CACHE
    if _NC_CACHE is None:
        _NC_CACHE = build_nc()
    return _NC_CACHE


def make_in_maps(wheel_speeds, remaining_clicks, converted, rw_signal_state):
    u8 = np.uint8
    ws = np.asarray(wheel_speeds, dtype=np.float32).reshape(N_CORES, NT, P, FD)
    rc = np.asarray(remaining_clicks, dtype=np.float32).reshape(N_CORES, NT, P, FD)
    rc_bf = np.asarray(remaining_clicks, dtype=np.float32).astype(
        ml_dtypes.bfloat16).view('<u2').astype('<u4')
    cv_bf = np.asarray(converted, dtype=np.float32).astype(
        ml_dtypes.bfloat16).view('<u2').astype('<u4')
    pd32 = (rc_bf | (cv_bf << 16)).view('<i4').reshape(N_CORES, NT, P, FD)
    st8 = np.asarray(rw_signal_state, dtype=np.int32).astype(np.int8)
    packed = np.concatenate(
        [
            ws.view(u8).reshape(N_CORES, NT, P, 4 * FD),
            rc.view(u8).reshape(N_CORES, NT, P, 4 * FD),
            st8.view(u8).reshape(N_CORES, NT, P, FD),
        ],
        axis=3,
    )  # [cores, nt, P, 9*FD]
    return [
        {
            "packed_main": np.ascontiguousarray(packed[c]),
            "packed_pd": np.ascontiguousarray(pd32[c]),
        }
        for c in range(N_CORES)
    ]


def unpack_results(results):
    po = np.stack(
        [np.asarray(results[c]["packed_out"]) for c in range(N_CORES)], axis=0
    )
    pairs = po.view('<u2').reshape(N_CORES, NT, P, FD, 2)
    rem = pairs[..., 0].copy().view(ml_dtypes.bfloat16).astype(np.float32)
    out = pairs[..., 1].copy().view(ml_dtypes.bfloat16).astype(np.float32)
    return out.reshape(N_TOTAL), rem.reshape(N_TOTAL)


def kernel(wheel_speeds, remaining_clicks, converted, rw_signal_state):
    nc = _get_nc()
    in_maps = make_in_maps(wheel_speeds, remaining_clicks, converted, rw_signal_state)
    res = run_bass_kernel_spmd(nc, in_maps, core_ids=list(range(N_CORES)))
    return unpack_results(res.results)


# revision 4
# speedup vs baseline: 1.0492x; 1.0297x over previous
"""Trainium2 Bass kernel for the reaction-wheel encoder elementwise problem (v3).

Reference semantics (per element, f32 unless noted):
    temp   = wheel_speeds * K + remaining_clicks        (K = DT * CPR, f32)
    clicks = trunc(temp)
    nominal_out = clicks * (1/K)
    nominal_rem = temp - clicks
    state == 0 (nominal): out = nominal_out, rem = nominal_rem
    state == 1 (off):     out = 0,           rem = 0
    state == 2 (stuck):   out = converted,   rem = remaining_clicks

v3 design (vs the 141.6us baseline):
  * HBM traffic 21 -> 15 B/elem: `converted` sent as bf16, outputs
    written as bf16 (pure relative rounding of final values; tolerance
    2e-2 >> bf16's 2^-9).
  * Measured HW op costs per [128,2048] tile: custom DVE op 2132ns,
    tensor_tensor 2054, copy_predicated(2fd) 4108, ACT activation 1812,
    Pool tensor_tensor 4195 (0.42 eff).  scalar_tensor_tensor measured
    2623ns (cost model's "2x_2p" f32 mode is NOT real) - avoided.
  * Concurrent GpSimd/Pool activity was measured to slow concurrent DVE
    custom ops 3-4x (SBUF contention from the Q7 software engine), so
    Pool is left idle.
  * The two outputs are stored as INTERLEAVED bf16 pairs [rem_i|out_i]
    so the stuck override is ONE fd-wide copy_predicated on the i32
    pair view (copy_predicated has no 16-bit fast mode; element count
    is what costs).  The override data [rc_bf16|cv_bf16] arrives from
    the host as one interleaved i32 plane, and the i32 mask is a single
    ACT op.  Work split:
      DVE : A temp=affine(ws,rc,K); B x=temp*m0 (tensor_tensor);
            C rem=REM_TRUNC(x,sgn) f32; D out=CLICKS_SCALE(x,rem)->ob
            bf16 (strided); PRED copy_predicated(ob_i32, m2_i32, pd_i32)
      ACT : m0=Relu(1-st) i8; m2=Relu(st-1) i32; sgn=Sign(temp) i8
            (masked lanes have d=0 so the unmasked sign is safe);
            CONV rem->ob strided bf16
  * Masked lanes: x=temp*m0 collapses to +-0 through the trunc chain so
    off lanes yield +-0 in both outputs; PRED fixes stuck lanes.

Layout per core: packed_main [nt,P,9fd] u8 rows = ws f32 | rc f32 |
st i8; packed_pd [nt,P,fd] i32 = (cv_bf16<<16)|rc_bf16; packed_out
[nt,P,fd] i32 = interleaved (rem_bf16, out_bf16) pairs.

trunc(x): rn=(x+1.5*2^23)-1.5*2^23 (RNE); d=x-rn; rem=d+(d*sgn<0)*sgn.
"""

import os
import sys

import numpy as np
import ml_dtypes

for _p in ("/opt/trn_rl_repo", os.path.expanduser("~/.axon_site/_ro/trn_rl_repo")):
    if os.path.isdir(_p) and _p not in sys.path:
        sys.path.insert(0, _p)

import concourse.bass as bass
import concourse.mybir as mybir
import concourse.dve_ops as dve_ops
from concourse.dve_spec import C0 as _C0
from concourse.dve_spec import Spec, Src0, Src1, Zero, lower, _has_src1
from concourse.dve_uop import DveOpSpec
from concourse.bass_utils import run_bass_kernel_spmd

N_TOTAL = 16_777_216
N_CORES = 8
PER_CORE = N_TOTAL // N_CORES  # 2,097,152
P = 128
FD = 2048
NT = PER_CORE // (P * FD)  # 8 tiles/core
BUFS_IN = 4

F32 = mybir.dt.float32
BF16 = mybir.dt.bfloat16
I8 = mybir.dt.int8
I16 = mybir.dt.int16
I32 = mybir.dt.int32
U8 = mybir.dt.uint8
ALU = mybir.AluOpType
ACT = mybir.ActivationFunctionType

K32 = np.float32(0.1 * (2048.0 / (2.0 * np.pi)))
INVK32 = np.float32(1.0) / K32
MAGIC = float(np.float32(1.5 * 2.0**23))


def _register_custom_op(name, spec):
    for op in dve_ops.OPS:
        if op.name == name:
            return op
    row = dve_ops._CUSTOM_DVE_ROW_BASE + len(dve_ops.OPS)
    assert row < 0x20
    dve_ops._SUB_OPCODE_FOR_NAME[name] = row
    shas = {}
    for ver in ("v3", "v4"):
        try:
            tmp = DveOpSpec(
                name=name, opcode=row, uops=lower(spec, ver=ver),
                rd1_en=_has_src1(spec),
            )
            shas[ver] = tmp.sha(ver)
        except Exception:
            pass
    op = dve_ops.DveOp(name, spec, subdim=False, uops_sha=shas)
    dve_ops.OPS.append(op)
    dve_ops.CUSTOM_DVE_SPECS[name] = spec
    return op


def _rem_trunc_ref(in0, in1, s0, s1, imm2):
    x = in0.astype(np.float32)
    sgn = in1.astype(np.float32)
    rn = ((x + np.float32(s0)) - np.float32(s0)).astype(np.float32)
    d = (x - rn).astype(np.float32)
    away = ((d * sgn).astype(np.float32) < 0).astype(np.float32)
    return (d + away * sgn).astype(np.float32)


_d = (Src0 - ((Src0 + _C0) - _C0))
REM_TRUNC = _register_custom_op(
    "REM_TRUNC_ANT",
    Spec(body=_d + ((_d * Src1) < Zero) * Src1, reference=_rem_trunc_ref),
)

CLICKS_SCALE = _register_custom_op(
    "CLICKS_SCALE_ANT",
    Spec(
        body=(Src0 - Src1) * _C0,
        reference=lambda in0, in1, s0, s1, imm2: (
            (in0.astype(np.float32) - in1.astype(np.float32)) * np.float32(s0)
        ).astype(np.float32),
    ),
)


def build_nc(nt: int = NT, fd: int = FD, splits=None) -> bass.Bass:
    if splits is None:
        splits = [2, 1, 1, 1, 1, 1, 1, 2]  # halves at both ends x2
    nc = bass.Bass()
    in_d = nc.dram_tensor("packed_main", [nt, P, 9 * fd], U8, kind="ExternalInput")
    pd_d = nc.dram_tensor("packed_pd", [nt, P, fd], I32, kind="ExternalInput")
    out_d = nc.dram_tensor("packed_out", [nt, P, fd], I32, kind="ExternalOutput")
    in_v, pd_v, out_v = in_d[:], pd_d[:], out_d[:]

    with nc.sbuf_tensor("t_in", [P, BUFS_IN, 9 * fd], U8) as t_in, \
         nc.sbuf_tensor("t_pd", [P, 4, fd], I32) as t_pd, \
         nc.sbuf_tensor("t_temp", [P, 2, fd], F32) as t_temp, \
         nc.sbuf_tensor("t_x", [P, 2, fd], F32) as t_x, \
         nc.sbuf_tensor("t_sgn", [P, 2, fd], I8) as t_sgn, \
         nc.sbuf_tensor("t_rem", [P, 2, fd], F32) as t_rem, \
         nc.sbuf_tensor("t_m0", [P, 2, fd], I8) as t_m0, \
         nc.sbuf_tensor("t_m2", [P, 3, fd], I8) as t_m2, \
         nc.sbuf_tensor("t_ob", [P, 3, 2 * fd], BF16) as t_ob, \
         nc.sbuf_tensor("t_neg1", [P, 1], F32) as t_neg1:
        s_in1 = [nc.semaphore(name=f"s_in1_{b}").__enter__() for b in range(BUFS_IN)]
        s_in2 = [nc.semaphore(name=f"s_in2_{b}").__enter__() for b in range(4)]
        s_out = [nc.semaphore(name=f"s_out{b}").__enter__() for b in range(3)]
        s_dve = nc.semaphore(name="s_dve").__enter__()
        s_act = nc.semaphore(name="s_act").__enter__()
        s_ini = nc.semaphore(name="s_ini").__enter__()

        sched = []
        for t in range(nt):
            k = splits[t]
            w = fd // k
            for j in range(k):
                sched.append((t, j * w, w))
        nv = len(sched)
        # Ticks per virtual iteration (DVE order A, C, D, B, PRED —
        # no drain/tick after A and D; B's tick covers A's completion):
        #   DVE (iters 0..nv+1, 3/iter): C(i-1)=3i+1, B(i)=3i+2,
        #       PRED(i-2)=3i+3
        #   ACT (iters 0..nv, 4/iter): m0(v)=4v+1, m2(v)=4v+2,
        #       sgn(v)=4v+3, conv(v-1)=4v+4
        ka = [0] * nv
        kb = [0] * nv
        cnt1 = [0] * BUFS_IN
        cnt2 = [0] * 4

        def dma_in1(v):
            t, c, w = sched[v]
            b = v % BUFS_IN
            if w == fd:
                nc.sync.dma_start(
                    t_in.ap()[:, b, :], in_v[t]
                ).then_inc(s_in1[b], 16)
                cnt1[b] += 1
            else:
                # ws+rc column chunk: two 4w-byte ranges at stride 4*fd
                src = in_v[t, :, 0 : 8 * fd].rearrange(
                    "p (a z) -> p a z", a=2
                )[:, :, 4 * c : 4 * c + 4 * w]
                dst = t_in.ap()[:, b, 0 : 8 * fd].rearrange(
                    "p (a z) -> p a z", a=2
                )[:, :, 4 * c : 4 * c + 4 * w]
                nc.sync.dma_start(dst, src).then_inc(s_in1[b], 16)
                nc.sync.dma_start(
                    t_in.ap()[:, b, 8 * fd + c : 8 * fd + c + w],
                    in_v[t, :, 8 * fd + c : 8 * fd + c + w],
                ).then_inc(s_in1[b], 16)
                cnt1[b] += 2
            ka[v] = 16 * cnt1[b]

        def dma_in2(v):
            t, c, w = sched[v]
            b3 = v % 4
            nc.sync.dma_start(
                t_pd.ap()[:, b3, c : c + w],
                pd_v[t, :, c : c + w],
            ).then_inc(s_in2[b3], 16)
            cnt2[b3] += 1
            kb[v] = 16 * cnt2[b3]

        def dma_in(v):
            dma_in1(v)
            dma_in2(v)

        def in_f32(b, byte_off, w4):
            return t_in.ap()[:, b, byte_off : byte_off + 4 * w4].bitcast(F32)

        # ---- SP queue -----------------------------------------------------
        for v in range(min(BUFS_IN, nv)):
            dma_in(v)
        for v in range(nv):
            t, c, w = sched[v]
            s = v % 3
            if v + BUFS_IN < nv:
                # t_in slot (v+BUFS_IN)%4 = v%4: readers A(v) on DVE and
                # the st read on ACT (m2(v)=4v+2)
                nc.sync.wait_ge(s_dve, 3 * v + 2)   # B(v), covers A(v)
                nc.sync.wait_ge(s_act, 4 * v + 2)   # m2(v)
                dma_in1(v + BUFS_IN)
            nc.sync.wait_ge(s_dve, 3 * (v + 2) + 3)  # PRED(v) done
            if w == fd:
                dst = out_v[t]
                src = t_ob.ap()[:, s].bitcast(I32)
            else:
                dst = out_v[t][:, c : c + w]
                src = t_ob.ap()[:, s].bitcast(I32)[:, c : c + w]
            nc.sync.dma_start(dst, src).then_inc(s_out[s], 16)
            if v + BUFS_IN < nv:
                dma_in2(v + BUFS_IN)

        # ---- DVE queue ----------------------------------------------------
        nc.vector.memset(t_neg1.ap(), -1.0)
        nc.vector.drain()
        nc.vector.nop().then_inc(s_ini, 1)
        for i in range(nv + 2):
            # A(i): temp = (ws*K) + rc   (no drain: C/D don't read temp,
            # and B(i) later in this iteration carries the tick)
            if i < nv:
                t, c, w = sched[i]
                si = i % BUFS_IN
                nc.vector.wait_ge(s_in1[si], ka[i])
                if i >= 2:
                    nc.vector.wait_ge(s_act, 4 * (i - 2) + 3)   # sgn(i-2)
                nc.vector.affine_then_add(
                    out=t_temp.ap()[:, i % 2, 0:w],
                    in0=in_f32(si, 4 * c, w),
                    in1=in_f32(si, 4 * fd + 4 * c, w),
                    scale=float(K32), bias=0.0,
                )
            j = i - 1
            if 0 <= j < nv:
                t, c, w = sched[j]
                # C(j): rem = REM_TRUNC(x, sgn)
                nc.vector.wait_ge(s_act, 4 * j + 3)   # sgn(j)
                nc.vector._custom_dve(
                    REM_TRUNC, out=t_rem.ap()[:, j % 2, 0:w],
                    in0=t_x.ap()[:, j % 2, 0:w],
                    in1=t_sgn.ap()[:, j % 2, 0:w], s0=MAGIC,
                )
                nc.vector.drain()
                nc.vector.nop().then_inc(s_dve, 1)  # 3i+1
                # D(j): out = CLICKS_SCALE(x, rem) -> ob bf16 pairs (no drain)
                if j >= 3:
                    nc.vector.wait_ge(s_out[j % 3], 16 * (j // 3))
                nc.vector._custom_dve(
                    CLICKS_SCALE,
                    out=t_ob.ap()[:, j % 3].rearrange(
                        "p (z a) -> p z a", a=2
                    )[:, c : c + w, 1],
                    in0=t_x.ap()[:, j % 2, 0:w],
                    in1=t_rem.ap()[:, j % 2, 0:w], s0=float(INVK32),
                )
            else:
                nc.vector.nop().then_inc(s_dve, 1)  # 3i+1
            # B(i): x = temp * m0
            if i < nv:
                t, c, w = sched[i]
                nc.vector.wait_ge(s_act, 4 * i + 1)  # m0(i)
                nc.vector.tensor_tensor(
                    out=t_x.ap()[:, i % 2, 0:w],
                    in0=t_temp.ap()[:, i % 2, 0:w],
                    in1=t_m0.ap()[:, i % 2, 0:w],
                    op=ALU.mult,
                )
                nc.vector.drain()
            nc.vector.nop().then_inc(s_dve, 1)  # 3i+2
            k = i - 2
            if 0 <= k < nv:
                t, c, w = sched[k]
                # PRED(k): stuck override on the i32 pair view
                nc.vector.wait_ge(s_act, 4 * (k + 1) + 4)  # conv(k)
                nc.vector.wait_ge(s_in2[k % 4], kb[k])  # pd(k)
                nc.vector.copy_predicated(
                    out=t_ob.ap()[:, k % 3].bitcast(I32)[:, c : c + w],
                    mask=t_m2.ap()[:, k % 3, c : c + w],
                    data=t_pd.ap()[:, k % 4, c : c + w],
                )
                nc.vector.drain()
            nc.vector.nop().then_inc(s_dve, 1)  # 3i+3

        # ---- ACT queue ----------------------------------------------------
        nc.scalar.wait_ge(s_ini, 1)
        for v in range(nv + 1):
            if v < nv:
                t, c, w = sched[v]
                si = v % BUFS_IN
                st = t_in.ap()[:, si, 8 * fd + c : 8 * fd + c + w].bitcast(I8)
                nc.scalar.wait_ge(s_in1[si], ka[v])
                if v >= 2:
                    nc.scalar.wait_ge(s_dve, 3 * (v - 2) + 2)  # B(v-2): m0 slot
                # m0(v) i8
                nc.scalar.activation(
                    t_m0.ap()[:, v % 2, 0:w], st, ACT.Relu, bias=1.0, scale=-1.0
                )
                nc.scalar.drain()
                nc.scalar.nop().then_inc(s_act, 1)  # 4v+1
                # m2(v) i32
                if v >= 3:
                    nc.scalar.wait_ge(s_dve, 3 * (v - 1) + 3)  # PRED(v-3): m2 slot
                nc.scalar.activation(
                    t_m2.ap()[:, v % 3, c : c + w],
                    st, ACT.Relu, bias=t_neg1.ap(), scale=1.0,
                )
                nc.scalar.drain()
                nc.scalar.nop().then_inc(s_act, 1)  # 4v+2
                # sgn(v) from temp
                nc.scalar.wait_ge(s_dve, 3 * v + 2)  # B(v) (temp final after A)
                nc.scalar.activation(
                    t_sgn.ap()[:, v % 2, 0:w], t_temp.ap()[:, v % 2, 0:w],
                    ACT.Sign, bias=0.0, scale=1.0,
                )
                nc.scalar.drain()
                nc.scalar.nop().then_inc(s_act, 1)  # 4v+3
            else:
                for _ in range(3):
                    nc.scalar.nop().then_inc(s_act, 1)
            u = v - 1
            if 0 <= u < nv:
                t, c, w = sched[u]
                # conv(u): rem f32 -> ob strided bf16 (pair slot 0)
                nc.scalar.wait_ge(s_dve, 3 * v + 1)  # C(u) emitted in DVE iter v
                if u >= 3:
                    nc.scalar.wait_ge(s_out[u % 3], 16 * (u // 3))
                nc.scalar.activation(
                    t_ob.ap()[:, u % 3].rearrange(
                        "p (z a) -> p z a", a=2
                    )[:, c : c + w, 0],
                    t_rem.ap()[:, u % 2, 0:w], ACT.Copy, bias=0.0, scale=1.0,
                )
                nc.scalar.drain()
            nc.scalar.nop().then_inc(s_act, 1)  # 4v+4

    mybir.codegen_inst_isa_subclasses(nc)
    nc.finalize()
    return nc


_NC_CACHE: bass.Bass | None = None


def _get_nc() -> bass.Bass:
    global _NC_CACHE
    if _NC_CACHE is None:
        _NC_CACHE = build_nc()
    return _NC_CACHE


def make_in_maps(wheel_speeds, remaining_clicks, converted, rw_signal_state):
    u8 = np.uint8
    ws = np.asarray(wheel_speeds, dtype=np.float32).reshape(N_CORES, NT, P, FD)
    rc = np.asarray(remaining_clicks, dtype=np.float32).reshape(N_CORES, NT, P, FD)
    rc_bf = np.asarray(remaining_clicks, dtype=np.float32).astype(
        ml_dtypes.bfloat16).view('<u2').astype('<u4')
    cv_bf = np.asarray(converted, dtype=np.float32).astype(
        ml_dtypes.bfloat16).view('<u2').astype('<u4')
    pd32 = (rc_bf | (cv_bf << 16)).view('<i4').reshape(N_CORES, NT, P, FD)
    st8 = np.asarray(rw_signal_state, dtype=np.int32).astype(np.int8)
    packed = np.concatenate(
        [
            ws.view(u8).reshape(N_CORES, NT, P, 4 * FD),
            rc.view(u8).reshape(N_CORES, NT, P, 4 * FD),
            st8.view(u8).reshape(N_CORES, NT, P, FD),
        ],
        axis=3,
    )  # [cores, nt, P, 9*FD]
    return [
        {
            "packed_main": np.ascontiguousarray(packed[c]),
            "packed_pd": np.ascontiguousarray(pd32[c]),
        }
        for c in range(N_CORES)
    ]


def unpack_results(results):
    po = np.stack(
        [np.asarray(results[c]["packed_out"]) for c in range(N_CORES)], axis=0
    )
    pairs = po.view('<u2').reshape(N_CORES, NT, P, FD, 2)
    rem = pairs[..., 0].copy().view(ml_dtypes.bfloat16).astype(np.float32)
    out = pairs[..., 1].copy().view(ml_dtypes.bfloat16).astype(np.float32)
    return out.reshape(N_TOTAL), rem.reshape(N_TOTAL)


def kernel(wheel_speeds, remaining_clicks, converted, rw_signal_state):
    nc = _get_nc()
    in_maps = make_in_maps(wheel_speeds, remaining_clicks, converted, rw_signal_state)
    res = run_bass_kernel_spmd(nc, in_maps, core_ids=list(range(N_CORES)))
    return unpack_results(res.results)
